# revision 12
# baseline (speedup 1.0000x reference)
"""Distributed Taylor-series diffusion kernel for Trainium2 (8 NeuronCores).

Computes out[:, c] = expm(-t[c] * L) @ x[:, c] via a truncated Taylor series
    y = sum_{k=0}^{K} (-t)^k L^k x / k!
with K = 3. Writing L = I - S (S symmetric, ||S|| ~ 0.8, entries ~5e-3), each
power step is z_{k+1} = z_k - S z_k: the identity part stays in fp32, so only
the small S-product runs in fp8 e4m3 (S pre-scaled x32 on host, rescaled in
the accumulation). Measured rel err vs the order-25 fp32 reference ~3e-3,
well under the 2e-2 gate (fp8 quantization, not series truncation, dominates).

Distribution: S is symmetric; core j holds the column block S[:, 768j:768j+768]
resident in SBUF as fp8 (4.7 MB), pre-permuted on host into 24 DoubleRow
contraction tiles [128, 2, 768] (256 contraction rows per matmul instruction,
2 rows/cycle). Each step core j computes the transposed shard
(S z)^T[:, block_j] = z^T S[:, block_j] in two 384-column PSUM halves, writes
z^T_next = z^T_prev - psum/32, and one full-width [16, 768] fp32 AllGather per
step (two half-AGs were measured to serialize on the CC engine; one wide AG is
cheaper). The gathered [128, 768] z^T is DMA'd back in 8 partition-splits,
block-transposed to natural layout on the DVE and converted to e4m3 for the
next step's stationary operand. Taylor coefficients are folded into a per-half
fp32 accumulator; the k=0 term (x itself) is added exactly on the host, and
the last step's accumulation is pre-folded so only one vector op and the
output DMA sit on the tail.
"""

import sys

sys.path.insert(0, "/opt/trn_rl_repo")

import numpy as np
import ml_dtypes

import concourse.bass as bass
import concourse.mybir as mybir
import concourse.tile as tile
from concourse import bacc
from concourse.bass_utils import run_bass_kernel_spmd

F32 = mybir.dt.float32
BF16 = mybir.dt.bfloat16
FP8 = mybir.dt.float8e4

V = 6144
C = 16
N_CORES = 8
VS = V // N_CORES          # 768 columns of S per core
HV = VS // 2               # 384: psum half width
K_STEPS = 3
NDT = 24                   # DoubleRow contraction tiles (256 rows each)
SSCALE = 32.0              # fp8 quantization scale for S
NTS = K_STEPS + 1          # coefficient columns (last = -c_K/SSCALE)

TRACE = False
LAST_RESULT = None

_cached_nc = None


def _v_index(d, P, i):
    """Global row v held by (dtile d, partition P, slot i)."""
    a = P // 32
    e = P % 32
    h = d // 12
    m = (d // 4) % 3
    q = d % 4
    return 768 * (2 * a + i) + 384 * h + 128 * m + 32 * q + e


def _build():
    nc = bacc.Bacc("TRN2", target_bir_lowering=False, debug=False,
                   num_devices=N_CORES)

    S_in = nc.dram_tensor("S8", [NDT, 128, 2 * VS], FP8, kind="ExternalInput")
    x8_in = nc.dram_tensor("x8", [128, NDT * 32], FP8, kind="ExternalInput")
    xt_in = nc.dram_tensor("xt", [C, VS], F32, kind="ExternalInput")
    ts_in = nc.dram_tensor("ts", [NTS, C], F32, kind="ExternalInput")
    out_d = nc.dram_tensor("out", [C, VS], F32, kind="ExternalOutput")

    rg = [list(range(N_CORES))]

    with tile.TileContext(nc) as tc:
        with (
            tc.tile_pool(name="Sp", bufs=1) as Sp,
            tc.tile_pool(name="natp", bufs=2) as natp,
            tc.tile_pool(name="natfp", bufs=2) as natfp,
            tc.tile_pool(name="ztp", bufs=2) as ztp,
            tc.tile_pool(name="znp", bufs=2) as znp,
            tc.tile_pool(name="smallp", bufs=1) as smallp,
            tc.tile_pool(name="psp", bufs=2, space="PSUM") as psp,
            tc.tile_pool(name="pswp", bufs=1, space="PSUM") as pswp,
            tc.tile_pool(name="dram", bufs=2, space="DRAM") as dram,
        ):
            # ---- coefficients ts_sb[c, k]; col K_STEPS holds -c_K/SSCALE
            ts_sb = smallp.tile([C, NTS], F32)
            nc.sync.dma_start(ts_sb[:], ts_in[:].rearrange("k c -> c k"))

            # ---- warm up the collective path ASAP (first collective pays a
            # large one-time setup; run it concurrently with the S load)
            w_in = dram.tile([2, C], F32, tag="warm_in")
            w_out = dram.tile([2 * N_CORES, C], F32, tag="warm_out",
                              addr_space="Shared")
            nc.sync.dma_start(w_in[:], ts_in[0:2, :])
            nc.gpsimd.collective_compute(
                "AllGather", mybir.AluOpType.bypass, replica_groups=rg,
                ins=[w_in.opt()], outs=[w_out.opt()],
            )

            # ---- z_0 = x in natural fp8 layout (host-prepped, full V)
            nat = natp.tile([128, NDT * 32], FP8, tag="nat", name="nat1")
            nc.sync.dma_start(nat[:], x8_in[:])

            # ---- own-shard x^T (fp32) for the step-1 identity part
            xt_sb = smallp.tile([C, VS], F32)
            nc.scalar.dma_start(xt_sb[:], xt_in[:])

            # ---- accumulators (acc through k=K-1; accp pre-folds c_K z_{K-1})
            acc = smallp.tile([C, VS], F32)
            nc.vector.memset(acc[:], 0.0)
            accp = smallp.tile([C, VS], F32)

            # ---- resident S in fp8: 24 DoubleRow tiles [128, 2, 768]
            St = []
            for d in range(NDT):
                st = Sp.tile([128, 2 * VS], FP8, tag=f"S{d}", name=f"S{d}")
                eng = nc.sync if d % 2 == 0 else nc.scalar
                eng.dma_start(st[:], S_in[d, :, :])
                St.append(st)

            def lhsT_view(nt, d):
                # [128, 2, 16] DoubleRow stationary slice of the nat tile
                return nt[:].rearrange("p (d i c) -> p d i c",
                                       d=NDT, i=2, c=C)[:, d, :, :]

            zn_prev = None
            for k in range(1, K_STEPS + 1):
                pss = [psp.tile([32, HV], F32, tag=f"ps{g}", name=f"ps{g}_{k}")
                       for g in range(2)]
                # 48 DoubleRow matmuls: input half h=0 tiles first so the
                # engine can start before the full gathered z is transposed
                for h in (0, 1):
                    for g in (0, 1):
                        for dd in range(12):
                            d = 12 * h + dd
                            idx = 12 * h + dd
                            nc.tensor.matmul(
                                pss[g][0:C, :],
                                lhsT_view(nat, d),
                                St[d][:].rearrange("p (i n) -> p i n",
                                                   i=2)[:, :, HV * g:
                                                        HV * (g + 1)],
                                start=(idx == 0), stop=(idx == NDT - 1),
                                perf_mode=mybir.MatmulPerfMode.DoubleRow,
                            )

                if k < K_STEPS:
                    zn = [znp.tile([C, HV], F32, tag=f"zn{g}",
                                   name=f"zn{g}_{k}") for g in range(2)]
                    nat_next = natp.tile([128, NDT * 32], FP8, tag="nat",
                                         name=f"nat{k + 1}")
                    b_in = dram.tile([C, VS], F32, tag="bin", name=f"bin{k}")
                    b_out = dram.tile([N_CORES * C, VS], F32, tag="bout",
                                      name=f"bout{k}", addr_space="Shared")
                    # z_next^T = z_prev^T - psum/SSCALE (critical path)
                    for g in (0, 1):
                        zprev = (xt_sb[:, HV * g:HV * (g + 1)] if k == 1
                                 else zn_prev[g][:, :])
                        nc.vector.scalar_tensor_tensor(
                            zn[g][:, :], pss[g][0:C, :], -1.0 / SSCALE, zprev,
                            op0=mybir.AluOpType.mult, op1=mybir.AluOpType.add,
                        )
                    for g in (0, 1):
                        eng = nc.sync if g == 0 else nc.scalar
                        eng.dma_start(b_in[:, HV * g:HV * (g + 1)],
                                      zn[g][:, :])
                    nc.gpsimd.collective_compute(
                        "AllGather", mybir.AluOpType.bypass, replica_groups=rg,
                        ins=[b_in.opt()], outs=[b_out.opt()],
                    )
                    # accumulate while the collective is in flight
                    for g in (0, 1):
                        nc.vector.scalar_tensor_tensor(
                            acc[:, HV * g:HV * (g + 1)], zn[g][:, :],
                            ts_sb[:, k - 1:k], acc[:, HV * g:HV * (g + 1)],
                            op0=mybir.AluOpType.mult, op1=mybir.AluOpType.add,
                        )
                    if k == K_STEPS - 1:
                        for g in (0, 1):
                            nc.vector.scalar_tensor_tensor(
                                accp[:, HV * g:HV * (g + 1)], zn[g][:, :],
                                ts_sb[:, k:k + 1], acc[:, HV * g:HV * (g + 1)],
                                op0=mybir.AluOpType.mult,
                                op1=mybir.AluOpType.add,
                            )
                    # gathered z^T [128 = 16r+c, 768], 8 partition-splits
                    zt = ztp.tile([128, VS], F32, tag="zt", name=f"zt{k}")
                    for s in range(N_CORES):
                        for g2 in (0, 1):
                            eng = nc.scalar if (2 * s + g2) % 2 == 0 else nc.sync
                            eng.dma_start(
                                zt[C * s:C * (s + 1), HV * g2:HV * (g2 + 1)],
                                b_out[C * s:C * (s + 1), HV * g2:HV * (g2 + 1)])
                    # block-transpose to natural layout (fp32), then fp8
                    natf = natfp.tile([128, VS], F32, tag="natf",
                                      name=f"natf{k}")
                    ztv = zt[:].rearrange("p (h m q e) -> p h m q e",
                                          h=2, m=3, q=4, e=32)
                    nfv = natf[:].rearrange("p (h m q w) -> p h m q w",
                                            h=2, m=3, q=4, w=32)
                    wscr = natfp.tile([128, 32], FP8, tag="wscr",
                                       name=f"wscr{k}")
                    nc.scalar.copy(wscr[:], zt[:, 0:32])
                    warm = pswp.tile([32, HV], F32, tag="warm",
                                     name=f"warm{k}")
                    wl = wscr[:].rearrange("p (i c) -> p i c", i=2)
                    for w in range(8):
                        nc.tensor.matmul(
                            warm[0:C, :], wl,
                            St[0][:].rearrange("p (i n) -> p i n",
                                               i=2)[:, :, 0:HV],
                            start=True, stop=True,
                            perf_mode=mybir.MatmulPerfMode.DoubleRow,
                        )
                    for h in (0, 1):
                        for q in range(4):
                            nc.vector.transpose(nfv[:, h, :, q, :],
                                                ztv[:, h, :, q, :])
                        nc.scalar.copy(
                            nat_next[:, 12 * 32 * h:12 * 32 * (h + 1)],
                            natf[:, 12 * 32 * h:12 * 32 * (h + 1)],
                        )
                    zn_prev = zn
                    nat = nat_next
                else:
                    # final step: out = accp - (c_K/SSCALE) psum, then DMA out
                    fin = [znp.tile([C, HV], F32, tag=f"zn{g}",
                                    name=f"fin{g}") for g in range(2)]
                    for g in (0, 1):
                        nc.vector.scalar_tensor_tensor(
                            fin[g][:, :], pss[g][0:C, :],
                            ts_sb[:, K_STEPS:K_STEPS + 1],
                            accp[:, HV * g:HV * (g + 1)],
                            op0=mybir.AluOpType.mult, op1=mybir.AluOpType.add,
                        )
                        eng = nc.sync if g == 0 else nc.scalar
                        eng.dma_start(out_d[:, HV * g:HV * (g + 1)],
                                      fin[g][:, :])

    nc.compile()
    return nc


def _get_nc():
    global _cached_nc
    if _cached_nc is None:
        _cached_nc = _build()
    return _cached_nc


def _host_prep(x, L, t):
    """Permute/quantize inputs into the kernel's layouts."""
    e4 = ml_dtypes.float8_e4m3

    # Taylor coefficients with the reference's rounding: c_k = c_{k-1}*(-t/k)
    tc_ = np.clip(t, 1e-8, None)
    cs = []
    cur = np.ones(C, np.float32)
    for k in range(1, K_STEPS + 1):
        cur = cur * (-tc_ / np.float32(k))
        cs.append(cur)
    cs.append(-cs[-1] / np.float32(SSCALE))   # aux col for the final fold
    ts = np.ascontiguousarray(np.stack(cs).astype(np.float32))

    # v index for (dtile, partition, slot)
    dd, PP, ii = np.meshgrid(np.arange(NDT), np.arange(128), np.arange(2),
                             indexing="ij")
    vidx = _v_index(dd, PP, ii)              # [24, 128, 2]

    # natural-layout fp8 x: x8[P, d*32 + i*16 + c] = x[v(d,P,i), c]
    xq = x.astype(e4)
    x8 = xq[vidx.transpose(1, 0, 2)].reshape(128, NDT * 32)
    x8 = np.ascontiguousarray(x8)

    xt = np.ascontiguousarray(x.T)           # [C, V] fp32

    in_maps = []
    for j in range(N_CORES):
        Sblk = -L[:, VS * j:VS * (j + 1)] * np.float32(SSCALE)
        idx = np.arange(VS)
        Sblk[VS * j + idx, idx] += np.float32(SSCALE)
        Sq = Sblk.astype(e4)                 # [V, 768] fp8
        S8 = Sq[vidx].reshape(NDT, 128, 2 * VS)
        in_maps.append({
            "S8": np.ascontiguousarray(S8),
            "x8": x8,
            "xt": np.ascontiguousarray(xt[:, VS * j:VS * (j + 1)]),
            "ts": ts,
        })
    return in_maps


def kernel(x: np.ndarray, L: np.ndarray, t: np.ndarray) -> np.ndarray:
    global LAST_RESULT
    x = np.ascontiguousarray(np.asarray(x, dtype=np.float32))
    L = np.asarray(L, dtype=np.float32)
    t = np.asarray(t, dtype=np.float32)
    assert x.shape == (V, C) and L.shape == (V, V) and t.shape == (C,)

    in_maps = _host_prep(x, L, t)
    nc = _get_nc()
    res = run_bass_kernel_spmd(nc, in_maps, core_ids=list(range(N_CORES)),
                               trace=TRACE)
    LAST_RESULT = res

    y = np.empty((V, C), dtype=np.float32)
    for j in range(N_CORES):
        y[VS * j:VS * (j + 1), :] = res.results[j]["out"].T
    return x + y


# revision 13
# speedup vs baseline: 1.1431x; 1.1431x over previous
"""Distributed Taylor-series diffusion kernel for Trainium2 (8 NeuronCores).

Computes out[:, c] = expm(-t[c] * L) @ x[:, c] via a truncated Taylor series
    y = sum_{k=0}^{K} (-t)^k L^k x / k!
with K = 3. Writing L = I - S (S symmetric, ||S|| ~ 0.8, entries ~5e-3), each
power step is z_{k+1} = z_k - S z_k: the identity part stays in fp32, so only
the small S-product runs in fp8 e4m3 (S pre-scaled x32 on host, rescaled in
the accumulation). Measured rel err vs the order-25 fp32 reference ~3e-3,
well under the 2e-2 gate (fp8 quantization, not series truncation, dominates).

Distribution: S is symmetric; core j holds the column block S[:, 768j:768j+768]
resident in SBUF as fp8 (4.7 MB), pre-permuted on host into 24 DoubleRow
contraction tiles [128, 2, 768] (256 contraction rows per matmul instruction,
2 rows/cycle). Each step core j computes the transposed shard
(S z)^T[:, block_j] = z^T S[:, block_j] in two 384-column PSUM halves, writes
z^T_next = z^T_prev - psum/32, and one full-width [16, 768] fp32 AllGather per
step (two half-AGs were measured to serialize on the CC engine; one wide AG is
cheaper). The gathered [128, 768] z^T is DMA'd back in 8 partition-splits,
block-transposed to natural layout on the DVE and converted to e4m3 for the
next step's stationary operand. Taylor coefficients are folded into a per-half
fp32 accumulator; the k=0 term (x itself) is added exactly on the host, and
the last step's accumulation is pre-folded so only one vector op and the
output DMA sit on the tail.
"""

import sys

sys.path.insert(0, "/opt/trn_rl_repo")

import numpy as np
import ml_dtypes

import concourse.bass as bass
import concourse.mybir as mybir
import concourse.tile as tile
from concourse import bacc
from concourse.bass_utils import run_bass_kernel_spmd

F32 = mybir.dt.float32
BF16 = mybir.dt.bfloat16
FP8 = mybir.dt.float8e4

V = 6144
C = 16
N_CORES = 8
VS = V // N_CORES          # 768 columns of S per core
HV = VS // 2               # 384: psum half width
K_STEPS = 3
NDT = 24                   # DoubleRow contraction tiles (256 rows each)
SSCALE = 32.0              # fp8 quantization scale for S
NTS = K_STEPS + 1          # coefficient columns (last = -c_K/SSCALE)

TRACE = False
LAST_RESULT = None

_cached_nc = None


def _v_index(d, P, i):
    """Global row v held by (dtile d, partition P, slot i)."""
    a = P // 32
    e = P % 32
    h = d // 12
    m = (d // 4) % 3
    q = d % 4
    return 768 * (2 * a + i) + 384 * h + 128 * m + 32 * q + e


def _build():
    nc = bacc.Bacc("TRN2", target_bir_lowering=False, debug=False,
                   num_devices=N_CORES)

    S_in = nc.dram_tensor("S8", [NDT, 128, 2 * VS], FP8, kind="ExternalInput")
    x8_in = nc.dram_tensor("x8", [128, NDT * 32], FP8, kind="ExternalInput")
    xt_in = nc.dram_tensor("xt", [C, VS], F32, kind="ExternalInput")
    ts_in = nc.dram_tensor("ts", [NTS, C], F32, kind="ExternalInput")
    out_d = nc.dram_tensor("out", [C, VS], F32, kind="ExternalOutput")

    rg = [list(range(N_CORES))]

    with tile.TileContext(nc) as tc:
        with (
            tc.tile_pool(name="Sp", bufs=1) as Sp,
            tc.tile_pool(name="natp", bufs=2) as natp,
            tc.tile_pool(name="natfp", bufs=2) as natfp,
            tc.tile_pool(name="ztp", bufs=2) as ztp,
            tc.tile_pool(name="znp", bufs=2) as znp,
            tc.tile_pool(name="smallp", bufs=1) as smallp,
            tc.tile_pool(name="psp", bufs=2, space="PSUM") as psp,
            tc.tile_pool(name="dram", bufs=2, space="DRAM") as dram,
        ):
            # ---- coefficients ts_sb[c, k]; col K_STEPS holds -c_K/SSCALE
            ts_sb = smallp.tile([C, NTS], F32)
            nc.sync.dma_start(ts_sb[:], ts_in[:].rearrange("k c -> c k"))

            # ---- warm up the collective path ASAP (first collective pays a
            # large one-time setup; run it concurrently with the S load)
            w_in = dram.tile([2, C], F32, tag="warm_in")
            w_out = dram.tile([2 * N_CORES, C], F32, tag="warm_out",
                              addr_space="Shared")
            nc.sync.dma_start(w_in[:], ts_in[0:2, :])
            nc.gpsimd.collective_compute(
                "AllGather", mybir.AluOpType.bypass, replica_groups=rg,
                ins=[w_in.opt()], outs=[w_out.opt()],
            )

            # ---- z_0 = x in natural fp8 layout (host-prepped, full V)
            nat = natp.tile([128, NDT * 32], FP8, tag="nat", name="nat1")
            nc.sync.dma_start(nat[:], x8_in[:])

            # ---- own-shard x^T (fp32) for the step-1 identity part
            xt_sb = smallp.tile([C, VS], F32)
            nc.scalar.dma_start(xt_sb[:], xt_in[:])

            # ---- accumulators (acc through k=K-1; accp pre-folds c_K z_{K-1})
            acc = smallp.tile([C, VS], F32)
            nc.vector.memset(acc[:], 0.0)
            accp = smallp.tile([C, VS], F32)

            # ---- resident S in fp8: 24 DoubleRow tiles [128, 2, 768]
            St = []
            for d in range(NDT):
                st = Sp.tile([128, 2 * VS], FP8, tag=f"S{d}", name=f"S{d}")
                eng = nc.sync if d % 2 == 0 else nc.scalar
                eng.dma_start(st[:], S_in[d, :, :])
                St.append(st)

            def lhsT_view(nt, d):
                # [128, 2, 16] DoubleRow stationary slice of the nat tile
                return nt[:].rearrange("p (d i c) -> p d i c",
                                       d=NDT, i=2, c=C)[:, d, :, :]

            zn_prev = None
            for k in range(1, K_STEPS + 1):
                pss = [psp.tile([32, HV], F32, tag=f"ps{g}", name=f"ps{g}_{k}")
                       for g in range(2)]
                # 48 DoubleRow matmuls: input half h=0 tiles first so the
                # engine can start before the full gathered z is transposed
                for h in (0, 1):
                    for g in (0, 1):
                        for dd in range(12):
                            d = 12 * h + dd
                            idx = 12 * h + dd
                            nc.tensor.matmul(
                                pss[g][0:C, :],
                                lhsT_view(nat, d),
                                St[d][:].rearrange("p (i n) -> p i n",
                                                   i=2)[:, :, HV * g:
                                                        HV * (g + 1)],
                                start=(idx == 0), stop=(idx == NDT - 1),
                                perf_mode=mybir.MatmulPerfMode.DoubleRow,
                            )

                if k < K_STEPS:
                    zn = [znp.tile([C, HV], F32, tag=f"zn{g}",
                                   name=f"zn{g}_{k}") for g in range(2)]
                    nat_next = natp.tile([128, NDT * 32], FP8, tag="nat",
                                         name=f"nat{k + 1}")
                    b_in = dram.tile([C, VS], F32, tag="bin", name=f"bin{k}")
                    b_out = dram.tile([N_CORES * C, VS], F32, tag="bout",
                                      name=f"bout{k}", addr_space="Shared")
                    # z_next^T = z_prev^T - psum/SSCALE (critical path)
                    for g in (0, 1):
                        zprev = (xt_sb[:, HV * g:HV * (g + 1)] if k == 1
                                 else zn_prev[g][:, :])
                        nc.vector.scalar_tensor_tensor(
                            zn[g][:, :], pss[g][0:C, :], -1.0 / SSCALE, zprev,
                            op0=mybir.AluOpType.mult, op1=mybir.AluOpType.add,
                        )
                    for g in (0, 1):
                        eng = nc.sync if g == 0 else nc.scalar
                        eng.dma_start(b_in[:, HV * g:HV * (g + 1)],
                                      zn[g][:, :])
                    nc.gpsimd.collective_compute(
                        "AllGather", mybir.AluOpType.bypass, replica_groups=rg,
                        ins=[b_in.opt()], outs=[b_out.opt()],
                    )
                    # accumulate while the collective is in flight
                    for g in (0, 1):
                        nc.vector.scalar_tensor_tensor(
                            acc[:, HV * g:HV * (g + 1)], zn[g][:, :],
                            ts_sb[:, k - 1:k], acc[:, HV * g:HV * (g + 1)],
                            op0=mybir.AluOpType.mult, op1=mybir.AluOpType.add,
                        )
                    if k == K_STEPS - 1:
                        for g in (0, 1):
                            nc.vector.scalar_tensor_tensor(
                                accp[:, HV * g:HV * (g + 1)], zn[g][:, :],
                                ts_sb[:, k:k + 1], acc[:, HV * g:HV * (g + 1)],
                                op0=mybir.AluOpType.mult,
                                op1=mybir.AluOpType.add,
                            )
                    # gathered z^T [128 = 16r+c, 768], 8 partition-splits
                    zt = ztp.tile([128, VS], F32, tag="zt", name=f"zt{k}")
                    for s in range(N_CORES):
                        eng = nc.scalar if s % 2 == 0 else nc.sync
                        eng.dma_start(zt[C * s:C * (s + 1), :],
                                      b_out[C * s:C * (s + 1), :])
                    # block-transpose to natural layout (fp32), then fp8
                    natf = natfp.tile([128, VS], F32, tag="natf",
                                      name=f"natf{k}")
                    ztv = zt[:].rearrange("p (h m q e) -> p h m q e",
                                          h=2, m=3, q=4, e=32)
                    nfv = natf[:].rearrange("p (h m q w) -> p h m q w",
                                            h=2, m=3, q=4, w=32)
                    for h in (0, 1):
                        for q in range(4):
                            nc.vector.transpose(nfv[:, h, :, q, :],
                                                ztv[:, h, :, q, :])
                        nc.scalar.copy(
                            nat_next[:, 12 * 32 * h:12 * 32 * (h + 1)],
                            natf[:, 12 * 32 * h:12 * 32 * (h + 1)],
                        )
                    zn_prev = zn
                    nat = nat_next
                else:
                    # final step: out = accp - (c_K/SSCALE) psum, then DMA out
                    fin = [znp.tile([C, HV], F32, tag=f"zn{g}",
                                    name=f"fin{g}") for g in range(2)]
                    for g in (0, 1):
                        nc.vector.scalar_tensor_tensor(
                            fin[g][:, :], pss[g][0:C, :],
                            ts_sb[:, K_STEPS:K_STEPS + 1],
                            accp[:, HV * g:HV * (g + 1)],
                            op0=mybir.AluOpType.mult, op1=mybir.AluOpType.add,
                        )
                        eng = nc.sync if g == 0 else nc.scalar
                        eng.dma_start(out_d[:, HV * g:HV * (g + 1)],
                                      fin[g][:, :])

    nc.compile()
    return nc


def _get_nc():
    global _cached_nc
    if _cached_nc is None:
        _cached_nc = _build()
    return _cached_nc


def _host_prep(x, L, t):
    """Permute/quantize inputs into the kernel's layouts."""
    e4 = ml_dtypes.float8_e4m3

    # Taylor coefficients with the reference's rounding: c_k = c_{k-1}*(-t/k)
    tc_ = np.clip(t, 1e-8, None)
    cs = []
    cur = np.ones(C, np.float32)
    for k in range(1, K_STEPS + 1):
        cur = cur * (-tc_ / np.float32(k))
        cs.append(cur)
    cs.append(-cs[-1] / np.float32(SSCALE))   # aux col for the final fold
    ts = np.ascontiguousarray(np.stack(cs).astype(np.float32))

    # v index for (dtile, partition, slot)
    dd, PP, ii = np.meshgrid(np.arange(NDT), np.arange(128), np.arange(2),
                             indexing="ij")
    vidx = _v_index(dd, PP, ii)              # [24, 128, 2]

    # natural-layout fp8 x: x8[P, d*32 + i*16 + c] = x[v(d,P,i), c]
    xq = x.astype(e4)
    x8 = xq[vidx.transpose(1, 0, 2)].reshape(128, NDT * 32)
    x8 = np.ascontiguousarray(x8)

    xt = np.ascontiguousarray(x.T)           # [C, V] fp32

    in_maps = []
    for j in range(N_CORES):
        Sblk = -L[:, VS * j:VS * (j + 1)] * np.float32(SSCALE)
        idx = np.arange(VS)
        Sblk[VS * j + idx, idx] += np.float32(SSCALE)
        Sq = Sblk.astype(e4)                 # [V, 768] fp8
        S8 = Sq[vidx].reshape(NDT, 128, 2 * VS)
        in_maps.append({
            "S8": np.ascontiguousarray(S8),
            "x8": x8,
            "xt": np.ascontiguousarray(xt[:, VS * j:VS * (j + 1)]),
            "ts": ts,
        })
    return in_maps


def kernel(x: np.ndarray, L: np.ndarray, t: np.ndarray) -> np.ndarray:
    global LAST_RESULT
    x = np.ascontiguousarray(np.asarray(x, dtype=np.float32))
    L = np.asarray(L, dtype=np.float32)
    t = np.asarray(t, dtype=np.float32)
    assert x.shape == (V, C) and L.shape == (V, V) and t.shape == (C,)

    in_maps = _host_prep(x, L, t)
    nc = _get_nc()
    res = run_bass_kernel_spmd(nc, in_maps, core_ids=list(range(N_CORES)),
                               trace=TRACE)
    LAST_RESULT = res

    y = np.empty((V, C), dtype=np.float32)
    for j in range(N_CORES):
        y[VS * j:VS * (j + 1), :] = res.results[j]["out"].T
    return x + y
